# revision 1
# baseline (speedup 1.0000x reference)
"""Trainium2 Bass kernel for DefaultKVCache attention (GQA decode-chunk).

Full-input contract: kernel(**inputs) takes the unsharded numpy inputs and
returns the full (B, NUM, H*HS) float32 output.

Problem shape (hardcoded):
  B=4, H=32, G=8 query groups (GQA 4 q-heads/group), HS=128,
  NUM=16 new tokens, cache length L=8192, input_pos (typically 4096).

Math: scatter key/value chunk into the kv-cache at input_pos, then causal
attention of the 16 new queries against cache[0:input_pos+16].  The cache
scatter never needs to materialize: rows [0,pos) come from k_cache/v_cache
and rows [pos,pos+16) come from key/value directly.  Only the last 16
columns need the causal mask (cache rows are visible to every query).

Sharding: (batch, group) pairs across 8 cores: core c -> b=c//2,
groups 4*(c%2)..4*(c%2)+4.  Fully local attention, no collectives.

Per-core pipeline (2 group-pairs, each stacking 2 groups on partitions):
  - SWDGE cast-DMA streams K/V f32->bf16 (1MB reads, 2KB contiguous runs;
    cache rows land in a permuted t-order, which is sound because attention
    is permutation-invariant along the key axis as long as K and V use the
    same order).
  - PE transposes K 128x128 tiles (bf16, via identity matmul).
  - QK: stationary qT (host-pretransposed bf16), moving K^T, PSUM f32.
  - ScalarE: exp(scale*s) PSUM->SBUF bf16 with per-chunk row-sum accumulation
    (softmax denominators; no max-subtraction needed: logits are ~N(0,1)).
  - PE transposes exp tiles; PV accumulates attn@V in PSUM f32.
  - VectorE: reciprocal of denominator, scale, store.
"""
import sys
import numpy as np

for _p in ("/opt/trn_rl_repo", "/root/.axon_site/_ro/trn_rl_repo"):
    if _p not in sys.path:
        sys.path.insert(0, _p)

import ml_dtypes
from contextlib import ExitStack

import jax
from jax.sharding import Mesh, PartitionSpec
from jax.experimental.shard_map import shard_map

import concourse.bass as bass
from concourse import bacc, mybir, tile
import concourse.bass2jax as b2j

DEBUG_OPTS = {}
B, H, G, HS = 4, 32, 8, 128
NUM = 16
N_CORES = 8
NG = 4            # groups per core
QI = 64           # queries per group (4 heads x 16 tokens)
CH = 512          # cache chunk (4 x 128 sub-tiles)
F32 = mybir.dt.float32
BF16 = mybir.dt.bfloat16
NEG = -1e30
EXP = mybir.ActivationFunctionType.Exp


def _chunk_plan(pos):
    """[(t0, nsub, rsub)]: 512-row chunks, the last one may be partial
    (nsub full 128-sub-tiles + rsub leftover rows)."""
    plan = []
    t0 = 0
    while t0 + CH <= pos:
        plan.append((t0, 4, 0))
        t0 += CH
    if pos - t0:
        plan.append((t0, (pos - t0) // 128, (pos - t0) % 128))
    return plan


def build_program(pos, stage=99):
    """stage: debug truncation level. 0=dma only, 1=+Ktransp, 2=+QK,
    3=+exp, 4=+attnT, 5=+PV, 6=+tail, 99=full."""
    nc = bacc.Bacc("TRN2", target_bir_lowering=False, debug=False,
                   enable_asserts=False, num_devices=N_CORES)
    kc = vc = None
    if pos:
        kc = nc.dram_tensor("kc", [NG, pos, HS], F32, kind="ExternalInput").ap()
        vc = nc.dram_tensor("vc", [NG, pos, HS], F32, kind="ExternalInput").ap()
    knew = nc.dram_tensor("knew", [NG, NUM, HS], F32, kind="ExternalInput").ap()
    vnew = nc.dram_tensor("vnew", [NG, NUM, HS], F32, kind="ExternalInput").ap()
    qT = nc.dram_tensor("qT", [NG, HS, QI], BF16, kind="ExternalInput").ap()
    ident = nc.dram_tensor("ident", [128, 128], BF16, kind="ExternalInput").ap()
    maskb = nc.dram_tensor("maskb", [128, NUM], F32, kind="ExternalInput").ap()
    out = nc.dram_tensor("out", [NG, QI, HS], F32, kind="ExternalOutput").ap()

    plan = _chunk_plan(pos)
    n_full = sum(1 for p_ in plan if p_[1] == 4 and p_[2] == 0)
    n_cols = len(plan) + 1          # denominator columns (chunks + new tail)
    scale = float(HS) ** -0.5

    with tile.TileContext(nc) as tc, ExitStack() as ctx:
        _b = DEBUG_OPTS.get
        cpool = ctx.enter_context(tc.tile_pool(name="consts", bufs=1))
        kvpool = ctx.enter_context(tc.tile_pool(name="kv", bufs=_b("kv_bufs", 10)))
        ktpool = ctx.enter_context(tc.tile_pool(name="kt", bufs=_b("kt_bufs", 3)))
        epool = ctx.enter_context(tc.tile_pool(name="exp", bufs=_b("e_bufs", 3)))
        etpool = ctx.enter_context(tc.tile_pool(name="expT", bufs=_b("et_bufs", 3)))
        dpool = ctx.enter_context(tc.tile_pool(name="den", bufs=2))
        fpool = ctx.enter_context(tc.tile_pool(name="fin", bufs=2))
        ps_kt = ctx.enter_context(tc.tile_pool(name="ps_kt", bufs=2, space="PSUM"))
        ps_sc = ctx.enter_context(tc.tile_pool(name="ps_sc", bufs=2, space="PSUM"))
        ps_et = ctx.enter_context(tc.tile_pool(name="ps_et", bufs=2, space="PSUM"))
        ps_o = ctx.enter_context(tc.tile_pool(name="ps_o", bufs=2, space="PSUM"))

        # constants
        id_sb = cpool.tile([128, 128], BF16, tag="id")
        nc.sync.dma_start(id_sb[:], ident[:])
        q_sb = cpool.tile([128, NG, QI], BF16, tag="q")
        nc.sync.dma_start(q_sb[:], qT.rearrange("g p q -> p g q"))
        mb_sb = cpool.tile([128, NUM], F32, tag="mb")
        nc.sync.dma_start(mb_sb[:], maskb[:])
        kn_sb = cpool.tile([NUM, NG, HS], BF16, tag="kn")
        nc.gpsimd.dma_start(kn_sb[:], knew.rearrange("g t h -> t g h"))
        vn_sb = cpool.tile([NUM, NG, HS], BF16, tag="vn")
        nc.gpsimd.dma_start(vn_sb[:], vnew.rearrange("g t h -> t g h"))

        for rep in range(DEBUG_OPTS.get("reps", 1)):
          for pa in range(2):
            ga = 2 * pa
            den = dpool.tile([128, n_cols], F32, tag="den")
            # one accumulator bank per group: HW start=True clears
            # has_written for the whole bank, so chains must not share one
            out_ps = [ps_o.tile([128, HS], F32, tag="o", name=f"out_ps{pa}_{gi}")
                      for gi in range(2)]

            # --- stage K/V: one 1MB-read DMA per (4-chunk block, group,
            # tensor); within a chunk t = t0 + 4p + n (2KB runs/partition) ---
            BCH = 4          # chunks per dma block
            kv_blk = {}      # (which, blk, gi) -> (tile, nch)
            kv_part = None   # (ktile, vtile) for the partial chunk
            for blk in range(0, n_full, BCH):
                nch = min(BCH, n_full - blk)
                for wi, src in enumerate((kc, vc)):
                    for gi in range(2):
                        t = kvpool.tile([128, nch, 4, HS], BF16, tag="kv",
                                        name=f"kv{pa}_{wi}_{blk}_{gi}")
                        ap = src[ga + gi,
                                 blk * CH:(blk + nch) * CH, :].rearrange(
                            "(c p n) h -> p c n h", p=128, n=4)
                        nc.gpsimd.dma_start(t[:], ap)
                        kv_blk[("k" if wi == 0 else "v", blk, gi)] = t
            if len(plan) > n_full:
                t0, nsub, rsub = plan[-1]
                tiles = []
                for src in (kc, vc):
                    tl = []
                    if nsub:
                        t = kvpool.tile([128, 2, nsub, HS], BF16, tag="kvp")
                        for gi in range(2):
                            ap = src[ga + gi, t0:t0 + nsub * 128, :].rearrange(
                                "(n p) h -> p n h", p=128)
                            nc.gpsimd.dma_start(t[:, gi], ap)
                        tl.append(t)
                    else:
                        tl.append(None)
                    if rsub:
                        t = kvpool.tile([128, 2, HS], BF16, tag="kvr")
                        nc.gpsimd.dma_start(
                            t[:rsub],
                            src[ga:ga + 2, t0 + nsub * 128:t0 + nsub * 128 + rsub,
                                :].rearrange("g p h -> p g h"))
                        tl.append(t)
                    else:
                        tl.append(None)
                    tiles.append(tl)
                kv_part = tiles

            first_pv = [True, True]

            def do_chunk(c, ncols_c, ksubs, vsubs, last=False):
                """ksubs/vsubs: list of (ap_fn(gi) -> AP [w,128], w)."""
                nsub_t = len(ksubs)
                if stage < 1:
                    return
                # K transpose -> PSUM -> SBUF (bf16 cast already done by DMA)
                kt_ps = ps_kt.tile([128, 1024], BF16, tag="kt")
                for gi in range(2):
                    off = 0
                    for (apf, w) in ksubs:
                        nc.tensor.transpose(
                            kt_ps[:, gi * 512 + off: gi * 512 + off + w],
                            apf(gi), id_sb[:w, :w])
                        off += w
                kt_sb = ktpool.tile([128, 1024], BF16, tag="kt")
                if ncols_c == CH:
                    nc.vector.tensor_copy(kt_sb[:, :], kt_ps[:, :])
                else:
                    for gi in range(2):
                        nc.vector.tensor_copy(
                            kt_sb[:, gi * 512: gi * 512 + ncols_c],
                            kt_ps[:, gi * 512: gi * 512 + ncols_c])
                if stage < 2:
                    return
                # QK (both groups stacked on PSUM partitions)
                sc_ps = ps_sc.tile([128, CH], F32, tag="sc")
                for gi in range(2):
                    nc.tensor.matmul(
                        sc_ps[64 * gi:64 * gi + 64, :ncols_c],
                        q_sb[:, ga + gi, :],
                        kt_sb[:, gi * 512: gi * 512 + ncols_c],
                        start=True, stop=True)
                if stage < 3:
                    return
                # exp + denominator accumulation
                e_sb = epool.tile([128, CH], BF16, tag="e")
                nc.scalar.activation(e_sb[:, :ncols_c], sc_ps[:, :ncols_c],
                                     EXP, scale=scale,
                                     accum_out=(den[:, c:c + 1]
                                                if stage >= 4 else None))
                if stage < 5:
                    return
                # attn transpose: one full 128-partition transpose per
                # 128-col block; output cols split into the two groups
                # (avoids base-partition-64 transpose, which faults on HW)
                et_ps = ps_et.tile([128, CH], BF16, tag="et")
                off = 0
                for (apf, w) in ksubs:
                    nc.tensor.transpose(
                        et_ps[:w, off: off + 128],
                        e_sb[:, off: off + w],
                        id_sb[:, :])
                    off += 128
                et_sb = etpool.tile([128, CH], BF16, tag="et")
                if ncols_c == CH:
                    nc.scalar.copy(et_sb[:, :], et_ps[:, :])
                else:
                    off = 0
                    for (apf, w) in ksubs:
                        nc.scalar.copy(et_sb[:w, off: off + 128],
                                       et_ps[:w, off: off + 128])
                        off += 128
                if stage < 6:
                    return
                # PV accumulate
                nv = len(vsubs)
                for gi in range(2):
                    for si, (apf, w) in enumerate(vsubs):
                        st = first_pv[gi] and si == 0
                        sp = last and si == nv - 1 and gi == 1
                        nc.tensor.matmul(
                            out_ps[gi][64 * gi:64 * gi + 64, :],
                            et_sb[:w, si * 128 + 64 * gi: si * 128 + 64 * gi + 64],
                            apf(gi),
                            start=st, stop=sp,
                            skip_group_check=not (st and gi == 0) and not sp)
                    first_pv[gi] = False

            # ---- new-token tail first (rows [pos, pos+NUM), causally
            # masked); its PVs open the accumulators so the final normalize
            # only waits on the last cache chunk ----
            if stage >= 7:
                ktt_ps = ps_kt.tile([128, 2 * NUM], BF16, tag="kt")
                for gi in range(2):
                    nc.tensor.transpose(ktt_ps[:, gi * NUM:(gi + 1) * NUM],
                                        kn_sb[:, ga + gi, :], id_sb[:NUM, :NUM])
                ktt_sb = ktpool.tile([128, 2 * NUM], BF16, tag="kt")
                nc.vector.tensor_copy(ktt_sb[:, :], ktt_ps[:, :])

                sct_ps = ps_sc.tile([128, NUM], F32, tag="sc")
                for gi in range(2):
                    nc.tensor.matmul(sct_ps[64 * gi:64 * gi + 64, :],
                                     q_sb[:, ga + gi, :],
                                     ktt_sb[:, gi * NUM:(gi + 1) * NUM],
                                     start=True, stop=True)
                nc.vector.tensor_add(sct_ps[:, :], sct_ps[:, :], mb_sb[:, :])

                ett_sb = epool.tile([128, NUM], BF16, tag="e")
                nc.scalar.activation(ett_sb[:, :], sct_ps[:, :], EXP,
                                     scale=scale,
                                     accum_out=den[:, n_cols - 1:n_cols])

                eTt_ps = ps_et.tile([NUM, 128], BF16, tag="et")
                nc.tensor.transpose(eTt_ps[:, :], ett_sb[:, :], id_sb[:, :])
                eTt_sb = etpool.tile([NUM, 128], BF16, tag="et")
                nc.scalar.copy(eTt_sb[:, :], eTt_ps[:, :])

                for gi in range(2):
                    nc.tensor.matmul(out_ps[gi][64 * gi:64 * gi + 64, :],
                                     eTt_sb[:, gi * 64:(gi + 1) * 64],
                                     vn_sb[:, ga + gi, :],
                                     start=True, stop=False)
                    first_pv[gi] = False

            ci_all = 0
            for c in range(n_full):
                blk, cb = (c // BCH) * BCH, c % BCH
                ksubs = [
                    (lambda gi, c_=cb, j_=j, b_=blk:
                     kv_blk[("k", b_, gi)][:, c_, j_, :], 128) for j in range(4)]
                vsubs = [
                    (lambda gi, c_=cb, j_=j, b_=blk:
                     kv_blk[("v", b_, gi)][:, c_, j_, :], 128) for j in range(4)]
                do_chunk(ci_all, CH, ksubs, vsubs,
                         last=(stage >= 7 and c == n_full - 1
                               and len(plan) == n_full))
                ci_all += 1
            if kv_part is not None:
                t0, nsub, rsub = plan[-1]
                (kp, kr), (vp, vr) = kv_part
                ksubs = [(lambda gi, j_=j: kp[:, gi, j_, :], 128)
                         for j in range(nsub)]
                vsubs = [(lambda gi, j_=j: vp[:, gi, j_, :], 128)
                         for j in range(nsub)]
                if rsub:
                    ksubs.append((lambda gi: kr[:rsub, gi, :], rsub))
                    vsubs.append((lambda gi: vr[:rsub, gi, :], rsub))
                do_chunk(ci_all, nsub * 128 + rsub, ksubs, vsubs,
                         last=(stage >= 7))
                ci_all += 1

            # close the accumulation groups (no-op adds on the final chunk
            # were already emitted with stop on the last PV above)
            # ---- normalize and store ----
            if stage < 7:
                o_sb = fpool.tile([128, HS], F32, tag="os")
                nc.vector.memset(o_sb[:, :], 0.0)
                nc.sync.dma_start(
                    out[ga:ga + 2].rearrange("g q h -> (g q) h"), o_sb[:, :])
                continue
            dtot = fpool.tile([128, 1], F32, tag="dt")
            nc.vector.reduce_sum(dtot[:, :], den[:, :], axis=mybir.AxisListType.X)
            rec = fpool.tile([128, 1], F32, tag="rc")
            nc.vector.reciprocal(rec[:, :], dtot[:, :])
            o_sb = fpool.tile([128, HS], F32, tag="os")
            for gi in range(2):
                sl = slice(64 * gi, 64 * gi + 64)
                nc.vector.tensor_scalar_mul(o_sb[sl, :], out_ps[gi][sl, :],
                                            rec[sl, :])
            nc.sync.dma_start(
                out[ga:ga + 2].rearrange("g q h -> (g q) h"), o_sb[:, :])

    nc.compile()
    return nc


class _Runner:
    def __init__(self, nc):
        b2j.install_neuronx_cc_hook()
        self.nc = nc
        in_names, out_names, out_avals, zero_outs = [], [], [], []
        for alloc in nc.m.functions[0].allocations:
            if not isinstance(alloc, mybir.MemoryLocationSet):
                continue
            name = alloc.memorylocations[0].name
            if alloc.kind == "ExternalInput":
                in_names.append(name)
            elif alloc.kind == "ExternalOutput":
                out_names.append(name)
                shape = tuple(alloc.tensor_shape)
                dtype = mybir.dt.np(alloc.dtype)
                out_avals.append(jax.core.ShapedArray(shape, dtype))
                zero_outs.append(np.zeros(shape, dtype))
        part = nc.partition_id_tensor.name if nc.partition_id_tensor else None
        if part is not None:
            in_names = [n for n in in_names if n != part]
        self.in_names, self.out_names = in_names, out_names
        self.out_avals, self.zero_outs = out_avals, zero_outs
        all_names = in_names + out_names + ([part] if part else [])
        n_params = len(in_names)

        def _body(*args):
            operands = list(args)
            if part is not None:
                operands.append(b2j.partition_id_tensor())
            return tuple(b2j._bass_exec_p.bind(
                *operands, out_avals=tuple(out_avals), in_names=tuple(all_names),
                out_names=tuple(out_names), lowering_input_output_aliases=(),
                sim_require_finite=True, sim_require_nnan=True, nc=nc))

        devices = jax.devices()[:N_CORES]
        self.mesh = Mesh(np.asarray(devices), ("core",))
        in_specs = (PartitionSpec("core"),) * (n_params + len(out_names))
        out_specs = (PartitionSpec("core"),) * len(out_names)
        self.fn = jax.jit(shard_map(_body, mesh=self.mesh, in_specs=in_specs,
                                    out_specs=out_specs, check_rep=False),
                          keep_unused=True)

    def run(self, in_maps):
        sharding = jax.sharding.NamedSharding(self.mesh, PartitionSpec("core"))
        args = []
        for name in self.in_names:
            arr = np.concatenate([np.asarray(m[name]) for m in in_maps], axis=0)
            args.append(jax.device_put(arr, sharding))
        for z in self.zero_outs:
            args.append(jax.device_put(
                np.zeros((N_CORES * z.shape[0], *z.shape[1:]), z.dtype), sharding))
        outs = self.fn(*args)
        jax.block_until_ready(outs)
        return [{name: np.asarray(outs[i]).reshape(
            N_CORES, *self.out_avals[i].shape)[c]
            for i, name in enumerate(self.out_names)}
            for c in range(N_CORES)]


_cache = {}


def _get_runner(pos):
    if pos not in _cache:
        _cache[pos] = _Runner(build_program(pos))
    return _cache[pos]


def _make_maskb():
    m = np.zeros((128, NUM), np.float32)
    for r in range(128):
        m[r, (r % NUM) + 1:] = NEG
    return m


def kernel(query, key, value, k_cache, v_cache, input_pos):
    query = np.asarray(query, np.float32)
    key = np.asarray(key, np.float32)
    value = np.asarray(value, np.float32)
    k_cache = np.asarray(k_cache, np.float32)
    v_cache = np.asarray(v_cache, np.float32)
    pos = int(input_pos)

    runner = _get_runner(pos)
    ident = np.eye(128, dtype=ml_dtypes.bfloat16)
    maskb = _make_maskb()

    in_maps = []
    for c in range(N_CORES):
        b = c // 2
        g0 = 4 * (c % 2)
        qs = query[b, g0 * 4:(g0 + NG) * 4]          # [16 heads, NUM, HS]
        qT = np.ascontiguousarray(
            qs.reshape(NG, QI, HS).transpose(0, 2, 1)).astype(ml_dtypes.bfloat16)
        m = {
            "knew": np.ascontiguousarray(key[b, g0:g0 + NG]),
            "vnew": np.ascontiguousarray(value[b, g0:g0 + NG]),
            "qT": qT, "ident": ident, "maskb": maskb,
        }
        if pos:
            m["kc"] = np.ascontiguousarray(k_cache[b, g0:g0 + NG, :pos])
            m["vc"] = np.ascontiguousarray(v_cache[b, g0:g0 + NG, :pos])
        in_maps.append(m)

    results = runner.run(in_maps)

    full = np.empty((B, H, NUM, HS), np.float32)
    for c in range(N_CORES):
        b = c // 2
        g0 = 4 * (c % 2)
        full[b, g0 * 4:(g0 + NG) * 4] = results[c]["out"].reshape(16, NUM, HS)
    return np.ascontiguousarray(
        full.transpose(0, 2, 1, 3).reshape(B, NUM, H * HS))

